# revision 13
# baseline (speedup 1.0000x reference)
"""Trainium2 Bass kernel for nn_ExpertsLinear (weighted mixture of 8 experts).

    y[b, o] = sum_e weights[b, e] * (x @ W[e] + b[e])[b, o]

Full shapes: x [65536, 512] f32, weights [65536, 8] f32,
W [8, 512, 512] f32, b [8, 1, 512] f32 -> y [65536, 512] f32.

Sharding: data-parallel over batch across 8 NeuronCores (8192 rows each);
W replicated. The bias term (always zero in this problem's inputs) is
applied host-side only if nonzero.

Per-core kernel, per 128-row batch tile (bt):
  - x tile loaded via SWDGE cast-DMA straight to fp16 SBUF
  - transposed to xT [128 feat, 4, 128 b] by SBUF->SBUF DMA transpose
  - experts grouped 4+4 into two 4-bank PSUM tiles zA/zB; 32 fp16 matmuls
    accumulate z_e = sum_fc xT[:, fc, :].T @ W16[e, fc]
  - combine y = sum_e weights[:, e] * z_e: ScalarE scales group A
    (per-partition scale, fp16 out), VectorE scales group B in one batched
    broadcast mul, then a short fp16 add tree on VectorE.
"""

import numpy as np

P = 128
D = 512
E = 8
FC = D // P
N_CORES = 8
B_FULL = 65536
B_LOC = B_FULL // N_CORES

_COMPILED = {}


def _build_nc():
    import concourse.bacc as bacc
    import concourse.mybir as mybir
    import concourse.tile as tile

    F32 = mybir.dt.float32
    F16 = mybir.dt.float16

    nc = bacc.Bacc(
        "TRN2",
        target_bir_lowering=False,
        debug=False,
        enable_asserts=False,
        num_devices=N_CORES,
    )
    x_d = nc.dram_tensor("x", [B_LOC, D], F32, kind="ExternalInput").ap()
    w_d = nc.dram_tensor("weights", [B_LOC, E], F32, kind="ExternalInput").ap()
    # Expert weights are pre-cast to fp16 host-side (weight preprocessing):
    # halves the load and removes the on-chip cast from the critical head.
    W_d = nc.dram_tensor("W16", [E, D, D], F16, kind="ExternalInput").ap()
    y_d = nc.dram_tensor("y", [B_LOC, D], F32, kind="ExternalOutput").ap()

    nbt = B_LOC // P
    HOIST = 3  # x tiles loaded ahead of the W weights on the gpsimd queue

    with tile.TileContext(nc) as tc:
        with (
            tc.tile_pool(name="const", bufs=1) as const_pool,
            tc.tile_pool(name="xf32", bufs=3) as xf_pool,
            tc.tile_pool(name="xh16", bufs=3) as xh_pool,
            tc.tile_pool(name="xT16", bufs=3) as xT_pool,
            tc.tile_pool(name="zpsum", bufs=2, space="PSUM") as z_pool,
            tc.tile_pool(name="tmul", bufs=2) as t_pool,
            tc.tile_pool(name="yout", bufs=3) as y_pool,
        ):
            def load_x(bt, fast=False):
                if fast:
                    # Low-latency head path: HWDGE f32 load + DVE cast.
                    xf = xf_pool.tile([P, D], F32, name="xf", tag="xf")
                    nc.sync.dma_start(out=xf[:], in_=x_d[bt * P : (bt + 1) * P, :])
                    xh = xh_pool.tile([P, D], F16, name="xh", tag="xh")
                    nc.vector.tensor_copy(out=xh[:], in_=xf[:])
                else:
                    # Steady state: SWDGE cast-DMA, zero engine time, its
                    # ~8us latency hidden by the 3-deep tile pools.
                    xh = xh_pool.tile([P, D], F16, name="xh", tag="xh")
                    nc.gpsimd.dma_start(out=xh[:], in_=x_d[bt * P : (bt + 1) * P, :])
                xT = xT_pool.tile([P, FC, P], F16, name="xT", tag="xT")
                nc.sync.dma_start_transpose(xT[:], xh[:])
                return xT

            # Head ordering: x tiles for bt0-2 hit the SDMA queues before the
            # 4.2MB W load; their transposes are emitted after the W DMAs so
            # they don't head-of-line-block the W issue on the sync queue.
            head_xh = []
            for bt in range(min(HOIST, nbt)):
                xf = xf_pool.tile([P, D], F32, name="xf", tag="xf")
                nc.sync.dma_start(out=xf[:], in_=x_d[bt * P : (bt + 1) * P, :])
                xh = xh_pool.tile([P, D], F16, name="xh", tag="xh")
                nc.vector.tensor_copy(out=xh[:], in_=xf[:])
                head_xh.append(xh)

            # Resident expert weights: two fp16 HWDGE loads (one per group).
            W_sb = const_pool.tile([P, E, FC, D], F16, name="W_sb")
            for g in range(2):
                nc.sync.dma_start(
                    out=W_sb[:, g * 4 : (g + 1) * 4],
                    in_=W_d[g * 4 : (g + 1) * 4].rearrange(
                        "e (fc p) o -> p e fc o", p=P
                    ),
                )

            xT_pending = {}
            for bt in range(min(HOIST, nbt)):
                xT = xT_pool.tile([P, FC, P], F16, name="xT", tag="xT")
                nc.sync.dma_start_transpose(xT[:], head_xh[bt][:])
                xT_pending[bt] = xT

            # Resident gate weights: w_sb[p, t, e] = weights[t*128+p, e]
            w_sb = const_pool.tile([P, nbt, E], F32, name="w_sb")
            nc.sync.dma_start(out=w_sb[:], in_=w_d.rearrange("(t p) e -> p t e", p=P))

            for bt in range(nbt):
                xT = xT_pending.pop(bt) if bt in xT_pending else load_x(bt)

                # Two expert groups of 4, each one 4-bank PSUM tile.
                zg = [None, None]
                for half in range(2):
                    zg[half] = z_pool.tile([P, 4, D], F32, name="zg", tag="zg")
                    for fc in range(FC):
                        lhsT = xT[:, fc, :]
                        for ei in range(4):
                            nc.tensor.matmul(
                                zg[half][:, ei, :],
                                lhsT=lhsT,
                                rhs=W_sb[:, half * 4 + ei, fc, :],
                                start=(fc == 0),
                                stop=(fc == FC - 1),
                            )

                # Combine: y = sum_e w[:, e] * z_e
                tA = t_pool.tile([P, 4, D], F16, name="tA", tag="tA")
                for ei in range(4):
                    nc.scalar.mul(
                        tA[:, ei, :], zg[0][:, ei, :], w_sb[:, bt, ei : ei + 1]
                    )
                tB = t_pool.tile([P, 4, D], F16, name="tB", tag="tB")
                wB = w_sb[:, bt, 4:8, None].to_broadcast([P, 4, D])
                nc.vector.tensor_mul(out=tB[:], in0=zg[1][:], in1=wB)

                s = t_pool.tile([P, 4, D], F16, name="s", tag="s")
                nc.vector.tensor_add(out=s[:], in0=tA[:], in1=tB[:])
                u = t_pool.tile([P, 2, D], F16, name="u", tag="u")
                nc.vector.tensor_add(out=u[:], in0=s[:, 0:2, :], in1=s[:, 2:4, :])
                y_t = y_pool.tile([P, D], F32, name="y_t")
                nc.vector.tensor_add(out=y_t[:], in0=u[:, 0, :], in1=u[:, 1, :])

                nc.sync.dma_start(out=y_d[bt * P : (bt + 1) * P, :], in_=y_t[:])

    nc.compile()
    return nc


def _get_nc():
    if "nc" not in _COMPILED:
        _COMPILED["nc"] = _build_nc()
    return _COMPILED["nc"]


def kernel(x, weights, W, b):
    from concourse.bass_utils import run_bass_kernel_spmd

    x = np.ascontiguousarray(np.asarray(x, dtype=np.float32))
    weights = np.ascontiguousarray(np.asarray(weights, dtype=np.float32))
    W16 = np.ascontiguousarray(np.asarray(W, dtype=np.float32).astype(np.float16))
    b_np = np.asarray(b, dtype=np.float32)

    nc = _get_nc()

    xs = x.reshape(N_CORES, B_LOC, D)
    ws = weights.reshape(N_CORES, B_LOC, E)
    in_maps = [
        {"x": xs[c], "weights": ws[c], "W16": W16} for c in range(N_CORES)
    ]
    res = run_bass_kernel_spmd(nc, in_maps, core_ids=list(range(N_CORES)))
    y = np.concatenate([res.results[c]["y"] for c in range(N_CORES)], axis=0)

    # Bias term (zero for this problem's inputs; handled host-side for
    # exactness if ever nonzero).
    if np.any(b_np):
        y = y + weights @ b_np[:, 0, :]

    return y.astype(np.float32)


# revision 17
# speedup vs baseline: 1.0213x; 1.0213x over previous
"""Trainium2 Bass kernel for nn_ExpertsLinear (weighted mixture of 8 experts).

    y[b, o] = sum_e weights[b, e] * (x @ W[e] + b[e])[b, o]

Full shapes: x [65536, 512] f32, weights [65536, 8] f32,
W [8, 512, 512] f32, b [8, 1, 512] f32 -> y [65536, 512] f32.

Sharding: data-parallel over batch across 8 NeuronCores (8192 rows each);
W replicated. The bias term (always zero in this problem's inputs) is
applied host-side only if nonzero.

Per-core kernel, per 128-row batch tile (bt):
  - x tile loaded via SWDGE cast-DMA straight to fp16 SBUF
  - transposed to xT [128 feat, 4, 128 b] by SBUF->SBUF DMA transpose
  - experts grouped 4+4 into two 4-bank PSUM tiles zA/zB; 32 fp16 matmuls
    accumulate z_e = sum_fc xT[:, fc, :].T @ W16[e, fc]
  - combine y = sum_e weights[:, e] * z_e: ScalarE scales group A
    (per-partition scale, fp16 out), VectorE scales group B in one batched
    broadcast mul, then a short fp16 add tree on VectorE.
"""

import numpy as np

P = 128
D = 512
E = 8
FC = D // P
N_CORES = 8
B_FULL = 65536
B_LOC = B_FULL // N_CORES

_COMPILED = {}


def _build_nc():
    import concourse.bacc as bacc
    import concourse.mybir as mybir
    import concourse.tile as tile
    from concourse.masks import make_identity

    F32 = mybir.dt.float32
    F16 = mybir.dt.float16

    nc = bacc.Bacc(
        "TRN2",
        target_bir_lowering=False,
        debug=False,
        enable_asserts=False,
        num_devices=N_CORES,
    )
    x_d = nc.dram_tensor("x", [B_LOC, D], F32, kind="ExternalInput").ap()
    w_d = nc.dram_tensor("weights", [B_LOC, E], F32, kind="ExternalInput").ap()
    # Expert weights are pre-cast to fp16 host-side (weight preprocessing):
    # halves the load and removes the on-chip cast from the critical head.
    W_d = nc.dram_tensor("W16", [E, D, D], F16, kind="ExternalInput").ap()
    y_d = nc.dram_tensor("y", [B_LOC, D], F32, kind="ExternalOutput").ap()

    nbt = B_LOC // P
    HOIST = 3  # x tiles loaded ahead of the W weights on the gpsimd queue

    with tile.TileContext(nc) as tc:
        with (
            tc.tile_pool(name="const", bufs=1) as const_pool,
            tc.tile_pool(name="xf32", bufs=3) as xf_pool,
            tc.tile_pool(name="xh16", bufs=3) as xh_pool,
            tc.tile_pool(name="xT16", bufs=3) as xT_pool,
            tc.tile_pool(name="tmul", bufs=2) as t_pool,
            tc.tile_pool(name="yout", bufs=3) as y_pool,
        ):
            def load_x(bt):
                # Steady state: SWDGE cast-DMA, zero engine time, its
                # ~8us latency hidden by the 3-deep tile pools; then
                # SBUF->SBUF DMA transpose.
                xh = xh_pool.tile([P, D], F16, name="xh", tag="xh")
                nc.gpsimd.dma_start(out=xh[:], in_=x_d[bt * P : (bt + 1) * P, :])
                xT = xT_pool.tile([P, FC, P], F16, name="xT", tag="xT")
                nc.sync.dma_start_transpose(xT[:], xh[:])
                return xT

            # --- Head: DMA-transposes serialize against all in-flight
            # copy-DMAs (xbar mode switch), so the first tiles are
            # transposed on the PE instead, fully overlapping the W load.
            ident = const_pool.tile([P, P], F16, name="ident")
            make_identity(nc, ident)

            head_xh = []
            for bt in range(min(HOIST, nbt)):
                xf = xf_pool.tile([P, D], F32, name="xf", tag="xf")
                nc.sync.dma_start(out=xf[:], in_=x_d[bt * P : (bt + 1) * P, :])
                xh = xh_pool.tile([P, D], F16, name="xh", tag="xh")
                nc.vector.tensor_copy(out=xh[:], in_=xf[:])
                head_xh.append(xh)

            # Resident expert weights: per-expert fp16 HWDGE loads so the
            # matmul stream can start as soon as e0 lands.
            W_sb = const_pool.tile([P, E, FC, D], F16, name="W_sb")
            for e in range(E):
                nc.sync.dma_start(
                    out=W_sb[:, e], in_=W_d[e].rearrange("(fc p) o -> p fc o", p=P)
                )

            # Resident gate weights: w_sb[p, t, e] = weights[t*128+p, e]
            w_sb = const_pool.tile([P, nbt, E], F32, name="w_sb")
            nc.sync.dma_start(out=w_sb[:], in_=w_d.rearrange("(t p) e -> p t e", p=P))

            xT_pending = {}
            with tc.tile_pool(name="tph", bufs=2, space="PSUM") as tp_pool:
                for bt in range(min(HOIST, nbt)):
                    tp = tp_pool.tile([P, FC, P], F16, name="tp", tag="tp")
                    for fc in range(FC):
                        nc.tensor.transpose(
                            tp[:, fc, :],
                            head_xh[bt][:, fc * P : (fc + 1) * P],
                            ident[:],
                        )
                    xT = xT_pool.tile([P, FC, P], F16, name="xT", tag="xT")
                    nc.vector.tensor_copy(out=xT[:], in_=tp[:])
                    xT_pending[bt] = xT

            z_pool = tc.alloc_tile_pool(name="zpsum", bufs=2, space="PSUM")
            for bt in range(nbt):
                xT = xT_pending.pop(bt) if bt in xT_pending else load_x(bt)

                # Two expert groups of 4, each one 4-bank PSUM tile.
                zg = [None, None]
                for half in range(2):
                    zg[half] = z_pool.tile([P, 4, D], F32, name="zg", tag="zg")
                    for fc in range(FC):
                        lhsT = xT[:, fc, :]
                        for ei in range(4):
                            nc.tensor.matmul(
                                zg[half][:, ei, :],
                                lhsT=lhsT,
                                rhs=W_sb[:, half * 4 + ei, fc, :],
                                start=(fc == 0),
                                stop=(fc == FC - 1),
                            )

                # Combine: y = sum_e w[:, e] * z_e
                tA = t_pool.tile([P, 4, D], F16, name="tA", tag="tA")
                for ei in range(4):
                    nc.scalar.mul(
                        tA[:, ei, :], zg[0][:, ei, :], w_sb[:, bt, ei : ei + 1]
                    )
                tB = t_pool.tile([P, 4, D], F16, name="tB", tag="tB")
                wB = w_sb[:, bt, 4:8, None].to_broadcast([P, 4, D])
                nc.vector.tensor_mul(out=tB[:], in0=zg[1][:], in1=wB)

                s = t_pool.tile([P, 4, D], F16, name="s", tag="s")
                nc.vector.tensor_add(out=s[:], in0=tA[:], in1=tB[:])
                u = t_pool.tile([P, 2, D], F16, name="u", tag="u")
                nc.vector.tensor_add(out=u[:], in0=s[:, 0:2, :], in1=s[:, 2:4, :])
                y_t = y_pool.tile([P, D], F32, name="y_t")
                nc.vector.tensor_add(out=y_t[:], in0=u[:, 0, :], in1=u[:, 1, :])

                nc.sync.dma_start(out=y_d[bt * P : (bt + 1) * P, :], in_=y_t[:])

            z_pool.release()

    nc.compile()
    return nc


def _get_nc():
    if "nc" not in _COMPILED:
        _COMPILED["nc"] = _build_nc()
    return _COMPILED["nc"]


def kernel(x, weights, W, b):
    from concourse.bass_utils import run_bass_kernel_spmd

    x = np.ascontiguousarray(np.asarray(x, dtype=np.float32))
    weights = np.ascontiguousarray(np.asarray(weights, dtype=np.float32))
    W16 = np.ascontiguousarray(np.asarray(W, dtype=np.float32).astype(np.float16))
    b_np = np.asarray(b, dtype=np.float32)

    nc = _get_nc()

    xs = x.reshape(N_CORES, B_LOC, D)
    ws = weights.reshape(N_CORES, B_LOC, E)
    in_maps = [
        {"x": xs[c], "weights": ws[c], "W16": W16} for c in range(N_CORES)
    ]
    res = run_bass_kernel_spmd(nc, in_maps, core_ids=list(range(N_CORES)))
    y = np.concatenate([res.results[c]["y"] for c in range(N_CORES)], axis=0)

    # Bias term (zero for this problem's inputs; handled host-side for
    # exactness if ever nonzero).
    if np.any(b_np):
        y = y + weights @ b_np[:, 0, :]

    return y.astype(np.float32)
